# revision 3
# baseline (speedup 1.0000x reference)
"""Trainium2 Bass kernel for MultiHeadAttention (B=2, S=4096, D=512, H=8).

Sharding: 16 (batch, head) units across 8 cores -> each core owns one batch
and a contiguous pair of heads (2 heads x 64 depth = 128 columns of the
QKV projections, 128 rows of the output projection).

v2 design (cost-model driven):
  * ScalarE is the hard floor: S*skc*2 exp evaluations can only run on the
    Activation engine (~1038ns per [128,1024] tile, 128 tiles => ~133us).
    Everything else is arranged to hide under that stream.
  * Activations ship as bf16 (halves DMA); all projections + scores are bf16
    matmuls (same 1.0 cycles/row as f32r in the cost model, enables 2-byte
    DVE modes downstream).
  * Scores land as [128 keys, 1024(=2 heads x 512 queries)] PSUM tiles; one
    exp per tile with a per-partition bias column that zeroes padded keys
    (bias -30) -- the mask multiply in V-assembly and the maskf input
    disappear.
  * exp writes bf16 P tiles [128, 2(head), 512].  (fp8 P for a DoubleRow
    A@V was measured at 2.8e-2 rel err on the fixed inputs -- the softmax
    is sharply peaked, so quantizing dominant attention weights does not
    average out.  bf16 keeps the error ~3e-3.)
  * V^T is produced directly by the projection (lhsT=x2, rhs=Wv cols), so
    the per-tile PE transposes and mask multiplies are gone.  V-augmented
    layout [128, 2(head), NPAIR, 2, 65] = [V(64) | 1] per key tile; the ones
    column (preset once by a gpsimd memset) yields the softmax denominator
    as row 64 of the same PSUM accumulation.
  * Normalization: reciprocal row -> gpsimd partition_broadcast (Pool engine,
    otherwise idle) -> two DVE muls into a head-stacked o_n [128, S] bf16,
    which lets the output projection contract over 128 partitions: one
    512-cycle matmul per 128-query tile instead of two.
  * ALL DMAs ride the SP HWDGE queue: issuing a DMA costs ~667ns on the
    issuing engine's sequencer, so outputs must NOT go through nc.scalar
    (that stalls the exp stream; measured 2.5us/chunk).
  * Emission order is engine execution order (in-order queues).  PE misc
    work (projections, output tiles, AV drains) is spread <=1 job per
    key-tile slot so the 2-buffer score ring never starves the exp stream.

Non-zero q/k/v biases or an all-masked batch fall back to a numpy reference
(cannot occur with the problem's setup_inputs).
"""

import numpy as np
import ml_dtypes

B, S, D, H = 2, 4096, 512, 8
DH = 64  # depth per head
NCORES = 8
C_SHIFT = 0.0  # bf16 P needs no range shift

_RUNTIMES = {}


def _build_program(skc: int, reps: int = 1):
    """Build the per-core Bass program. skc = padded compressed key count
    (multiple of 128)."""
    import concourse.bacc as bacc
    import concourse.mybir as mybir
    from concourse.tile import TileContext

    f32 = mybir.dt.float32
    f32r = mybir.dt.float32r
    bf16 = mybir.dt.bfloat16
    EXP = mybir.ActivationFunctionType.Exp

    NT = skc // 128  # key tiles
    NQC = S // 512  # query chunks (512 wide)
    NKC = (skc + 511) // 512  # key chunks for the K/V projections

    nc = bacc.Bacc("TRN2", target_bir_lowering=False, debug=False, num_devices=NCORES)

    x1t = nc.dram_tensor("x1t", [D, S], bf16, kind="ExternalInput")
    x2ct = nc.dram_tensor("x2ct", [D, skc], bf16, kind="ExternalInput")
    biasc = nc.dram_tensor("biasc", [128, NT], f32, kind="ExternalInput")
    wqkv = nc.dram_tensor("wqkv", [D, 384], bf16, kind="ExternalInput")
    wo = nc.dram_tensor("wo", [128, D], bf16, kind="ExternalInput")
    out = nc.dram_tensor("out", [S, D], f32, kind="ExternalOutput")

    with nc.allow_low_precision(
        reason="bf16/fp8 data with fp32 PSUM accumulation; validated 3e-3 rel err"
    ), TileContext(nc) as tc:
        with (
            tc.tile_pool(name="consts", bufs=1) as consts,
            tc.tile_pool(name="bigsb", bufs=1) as bigsb,
            tc.tile_pool(name="xstream", bufs=3) as xstream,
            tc.tile_pool(name="pexp", bufs=8) as pexp,
            tc.tile_pool(name="work", bufs=3) as work,
            tc.tile_pool(name="ps_sc", bufs=2, space="PSUM") as ps_sc,
            tc.tile_pool(name="ps_oacc", bufs=2, space="PSUM") as ps_oacc,
            tc.tile_pool(name="ps_misc", bufs=2, space="PSUM") as ps_misc,
        ):
            # ---- constants / persistent buffers (DMA issue order matters:
            # the DMA device drains them in order) ----
            # Startup DMAs split across BOTH HWDGE queues (each queue drains
            # one transfer at a time).  The Activation queue is free before
            # the first exp, so its sequencer cost is harmless here; all
            # steady-state DMAs stay on the SP queue.
            x1r = x1t.rearrange("(t p) s -> p t s", p=128)
            x2all = bigsb.tile([128, 4, skc], bf16)
            x2r = x2ct.rearrange("(t p) s -> p t s", p=128)
            wqkv_sb = consts.tile([128, 4, 384], bf16)
            nc.sync.dma_start(
                out=wqkv_sb, in_=wqkv.rearrange("(t p) m -> p t m", p=128)
            )
            wq_sb = wqkv_sb[:, :, 0:128]
            wk_sb = wqkv_sb[:, :, 128:256]
            wv_sb = wqkv_sb[:, :, 256:384]
            x1c0 = xstream.tile([128, 4, 512], bf16, tag="xs")
            nc.sync.dma_start(out=x1c0, in_=x1r[:, :, 0:512])
            biasc_sb = consts.tile([128, NT], f32)
            nc.sync.dma_start(out=biasc_sb, in_=biasc[:, :])
            nc.sync.dma_start(out=x2all[:, :, 0:128], in_=x2r[:, :, 0:128])
            nc.sync.dma_start(out=x2all[:, :, 128:512], in_=x2r[:, :, 128:512])
            for p0 in range(512, skc, 256):
                p1 = min(p0 + 256, skc)
                nc.sync.dma_start(
                    out=x2all[:, :, p0:p1], in_=x2r[:, :, p0:p1]
                )
            wo_sb = consts.tile([128, 512], bf16)
            nc.sync.dma_start(out=wo_sb, in_=wo[:, :])
            ones_bf = consts.tile([1, 256], bf16)
            nc.vector.memset(ones_bf, 1.0)
            ones_f = consts.tile([1, 64], f32)
            nc.vector.memset(ones_f, 1.0)
            ones64 = consts.tile([1, 64], f32r)
            nc.vector.tensor_copy(ones64, ones_f)

            # ---- persistent activations ----
            q_t = bigsb.tile([128, S], bf16)
            k_t = bigsb.tile([128, skc], bf16)
            # V-augmented, head-separated: [V(64) | 1] per key tile
            vaug = bigsb.tile([128, 2, NT, 65], bf16)
            o_n = bigsb.tile([128, S], bf16)

            warm = bigsb.tile([1, 1], f32)

            for _rep in range(reps):
                nc.gpsimd.memset(vaug, 1.0)  # presets the denominator columns
                # dummy activation: hoists the exp table load off the
                # critical first-exp path (costs ~190ns at t=0)
                nc.scalar.activation(
                    out=warm, in_=ones_f[0:1, 0:1], func=EXP, scale=1.0
                )

                def emit_kv_k(c2, lo=0, half=None, box=[None]):
                    """K_T projection for key-chunk c2, key cols [lo:end).
                    half=0/1 splits the 4-kt accumulation into two PE jobs."""
                    hi = min(512, skc - c2 * 512)
                    ks = slice(c2 * 512 + lo, c2 * 512 + hi)
                    cw = hi - lo
                    if half in (None, 0):
                        box[0] = ps_misc.tile([128, 512], f32, tag="misc", name="psk")
                    psk = box[0]
                    kts = range(4) if half is None else range(2 * half, 2 * half + 2)
                    for kt in kts:
                        nc.tensor.matmul(
                            psk[:, :cw],
                            wk_sb[:, kt, :],
                            x2all[:, kt, ks],
                            start=(kt == 0),
                            stop=(kt == 3),
                        )
                    if half in (None, 1):
                        nc.vector.tensor_copy(k_t[:, ks], psk[:, :cw])

                def emit_kv_v(t):
                    """V^T projection + vaug assembly for key tile t."""
                    ts = slice(t * 128, (t + 1) * 128)
                    psv = ps_misc.tile([128, 128], f32, tag="misc", name="psv")
                    for kt in range(4):
                        nc.tensor.matmul(
                            psv,
                            x2all[:, kt, ts],
                            wv_sb[:, kt, :],
                            start=(kt == 0),
                            stop=(kt == 3),
                        )
                    nc.vector.tensor_copy(vaug[:, 0, t, 0:64], psv[:, 0:64])
                    nc.vector.tensor_copy(vaug[:, 1, t, 0:64], psv[:, 64:128])

                def x1_fetch(c):
                    x1c = xstream.tile([128, 4, 512], bf16, tag="xs", name="x1c")
                    nc.sync.dma_start(out=x1c, in_=x1r[:, :, c * 512 : (c + 1) * 512])
                    return x1c

                def emit_qproj_mm(c, x1c, kt, psq_box, scalar_copy=False):
                    if kt == 0:
                        psq_box[0] = ps_misc.tile(
                            [128, 512], f32, tag="misc", name="psq"
                        )
                    nc.tensor.matmul(
                        psq_box[0],
                        wq_sb[:, kt, :],
                        x1c[:, kt, :],
                        start=(kt == 0),
                        stop=(kt == 3),
                    )
                    if kt == 3:
                        if scalar_copy:
                            nc.scalar.activation(
                                out=q_t[:, c * 512 : (c + 1) * 512],
                                in_=psq_box[0],
                                func=mybir.ActivationFunctionType.Copy,
                            )
                        else:
                            nc.vector.tensor_copy(
                                q_t[:, c * 512 : (c + 1) * 512], psq_box[0]
                            )

                def emit_av(oacc0, oacc1, t, pt):
                    for h, oacc in ((0, oacc0), (1, oacc1)):
                        nc.tensor.matmul(
                            oacc,
                            vaug[:, h, t, :],
                            pt[:, h, :],
                            start=(t == 0),
                            stop=(t == NT - 1),
                        )

                def emit_norm(c, oacc0, oacc1):
                    """recip row + Pool partition_broadcast + stacked o_n muls
                    (DVE/Pool only -- no PE work on the critical stream)."""
                    qs = slice(c * 512, (c + 1) * 512)
                    recip = work.tile([1, 1024], f32, tag="recip", bufs=2)
                    nc.vector.reciprocal(recip[:, 0:512], oacc0[64:65, :])
                    nc.vector.reciprocal(recip[:, 512:1024], oacc1[64:65, :])
                    rb = work.tile([64, 1024], f32, tag="rb", bufs=2)
                    nc.gpsimd.partition_broadcast(rb, recip)
                    nc.vector.tensor_mul(o_n[0:64, qs], oacc0[0:64, :], rb[:, 0:512])
                    nc.vector.tensor_mul(
                        o_n[64:128, qs], oacc1[0:64, :], rb[:, 512:1024]
                    )

                def emit_tp(st):
                    ss = slice(st * 128, (st + 1) * 128)
                    tp = ps_misc.tile([128, 512], f32, tag="misc", name="tp")
                    nc.tensor.matmul(tp, o_n[:, ss], wo_sb, start=True, stop=True)
                    out_sb = work.tile([128, 512], f32, tag="outsb", bufs=4)
                    nc.vector.tensor_copy(out_sb, tp)
                    nc.sync.dma_start(out=out[ss, :], in_=out_sb)

                def emit_scores_exp(c, t, pt):
                    qs_c = slice(c * 512, (c + 1) * 512)
                    sc = ps_sc.tile([128, 1024], f32, tag="sc", name="sc")
                    nc.tensor.matmul(
                        sc[:, 0:512],
                        k_t[0:64, t * 128 : (t + 1) * 128],
                        q_t[0:64, qs_c],
                        start=True,
                        stop=True,
                    )
                    nc.tensor.matmul(
                        sc[:, 512:1024],
                        k_t[64:128, t * 128 : (t + 1) * 128],
                        q_t[64:128, qs_c],
                        start=True,
                        stop=True,
                    )
                    nc.scalar.activation(
                        out=pt[:, :, :],
                        in_=sc,
                        func=EXP,
                        scale=0.125,
                        bias=biasc_sb[:, t : t + 1],
                    )

                # ---- PE p-state warmup: dependency-free matmuls keep the
                # PE continuously busy from ~1.2us until the first projection
                # inputs land (~5.4us), so the projections and first scores
                # run at full clock instead of the 2x-slow ramp p-state ----
                warm_ps = ps_sc.tile([128, 1024], f32, tag="sc", name="warm")
                for _w in range(20):
                    nc.tensor.matmul(
                        warm_ps[0:64, 0:256],
                        ones_bf[:, 0:64],
                        ones_bf,
                        start=True,
                        stop=True,
                    )

                # ---- prologue: Q(0) + first K tile, queue the rest ----
                x1cur = x1c0 if _rep == 0 else x1_fetch(0)
                psq_box = [None]
                for kt in range(4):
                    emit_qproj_mm(0, x1cur, kt, psq_box, scalar_copy=True)
                psk0 = ps_misc.tile([128, 128], f32, tag="misc", name="psk0")
                for kt in range(4):
                    nc.tensor.matmul(
                        psk0,
                        wk_sb[:, kt, :],
                        x2all[:, kt, 0:128],
                        start=(kt == 0),
                        stop=(kt == 3),
                    )
                nc.vector.tensor_copy(k_t[:, 0:128], psk0)

                # deferred PE jobs: (min_slot, fn), drained in order once
                # t >= min_slot; <=2 per slot in chunk 0, <=1 afterwards
                miscq = []
                miscq.append((0, lambda: emit_kv_k(0, lo=128)))
                for kc in range(1, NKC):
                    bx = [None]
                    miscq.append((0, lambda kc=kc, bx=bx: emit_kv_k(kc, half=0, box=bx)))
                    miscq.append((0, lambda kc=kc, bx=bx: emit_kv_k(kc, half=1, box=bx)))
                for tt in range(0, 2):
                    miscq.append((0, lambda tt=tt: emit_kv_v(tt)))
                for tt in range(2, NT):
                    miscq.append((1, lambda tt=tt: emit_kv_v(tt)))

                prev_chunk = None  # (c, oacc0, oacc1) not yet normalized
                pending = []  # [(oacc0, oacc1, t, pt)] w/o AV emitted yet
                for c in range(NQC):
                    oacc0 = ps_oacc.tile([65, 512], f32, tag="oacc", name="oacc0")
                    oacc1 = ps_oacc.tile([65, 512], f32, tag="oacc", name="oacc1")
                    if c + 1 < NQC:
                        x1next = x1_fetch(c + 1)  # DMA out now, used at t>=11

                    for t in range(NT):
                        pt = pexp.tile([128, 2, 512], bf16, tag="pt", name="pt")
                        emit_scores_exp(c, t, pt)
                        pending.append((oacc0, oacc1, t, pt))
                        # finish the previous chunk's AVs in the first slots
                        dr = 0
                        while pending and pending[0][0] is not oacc0 and dr < 2:
                            emit_av(*pending.pop(0))
                            dr += 1
                        if t == 1 and prev_chunk is not None:
                            emit_norm(*prev_chunk)
                            nc_ = prev_chunk[0]
                            for st in range(4 * nc_, 4 * nc_ + 4):
                                miscq.append((7, lambda st=st: emit_tp(st)))
                            prev_chunk = None
                        if t == 2 and c + 1 < NQC:
                            pb = [None]
                            for kt in range(4):
                                miscq.append(
                                    (11, lambda kt=kt, c=c, x=x1next, pb=pb:
                                        emit_qproj_mm(c + 1, x, kt, pb))
                                )
                        # own-chunk AVs lag until the oacc WAR with the
                        # previous chunk's norm has cleared
                        if t >= 9:
                            lag = 1 if c == NQC - 1 else 3
                            dr = 0
                            while len(pending) > lag and dr < 2:
                                emit_av(*pending.pop(0))
                                dr += 1
                        # at most one deferred job per slot (two in chunk 0)
                        lim = 2 if c == 0 and t < 7 else 1
                        if t == NT - 1:
                            lim = len(miscq)  # jobs must not cross the chunk
                        dj = 0
                        while miscq and miscq[0][0] <= t and dj < lim:
                            miscq.pop(0)[1]()
                            dj += 1
                    prev_chunk = (c, oacc0, oacc1)

                # ---- tail: drain, then final chunk norm + projection with a
                # PE broadcast (PE is idle here; skips the Pool launch+sems)
                while pending:
                    emit_av(*pending.pop(0))
                while miscq:
                    miscq.pop(0)[1]()
                # final-chunk norm: ScalarE is idle now -- oacc copies go on
                # the scalar engine in parallel with the DVE reciprocals, the
                # broadcast runs on the PE, and the four output-tile copies
                # alternate scalar/DVE.
                CPY = mybir.ActivationFunctionType.Copy
                cl, oacc0, oacc1 = prev_chunk
                qs = slice(cl * 512, (cl + 1) * 512)
                os0 = work.tile([64, 512], bf16, tag="osb", bufs=2)
                nc.scalar.activation(out=os0, in_=oacc0[0:64, :], func=CPY)
                os1 = work.tile([64, 512], bf16, tag="osb", bufs=2)
                nc.scalar.activation(out=os1, in_=oacc1[0:64, :], func=CPY)
                recipr = work.tile([1, 1024], f32r, tag="recipr")
                nc.vector.reciprocal(recipr[:, 0:512], oacc0[64:65, :])
                nc.vector.reciprocal(recipr[:, 512:1024], oacc1[64:65, :])
                rb0 = ps_misc.tile([128, 512], f32, tag="misc", name="rb0")
                nc.tensor.matmul(
                    rb0[0:64, :], ones64, recipr[:, 0:512], start=True, stop=True
                )
                rb1 = ps_misc.tile([128, 512], f32, tag="misc", name="rb1")
                nc.tensor.matmul(
                    rb1[0:64, :], ones64, recipr[:, 512:1024], start=True, stop=True
                )
                nc.vector.tensor_mul(o_n[0:64, qs], os0, rb0[0:64, :])
                nc.vector.tensor_mul(o_n[64:128, qs], os1, rb1[0:64, :])
                for j, st in enumerate(range(4 * cl, 4 * cl + 4)):
                    ss = slice(st * 128, (st + 1) * 128)
                    tp = ps_misc.tile([128, 512], f32, tag="misc", name="tp")
                    nc.tensor.matmul(tp, o_n[:, ss], wo_sb, start=True, stop=True)
                    out_sb = work.tile([128, 512], f32, tag="outsb", bufs=4)
                    if j % 2 == 0:
                        nc.scalar.activation(out=out_sb, in_=tp, func=CPY)
                    else:
                        nc.vector.tensor_copy(out_sb, tp)
                    nc.sync.dma_start(out=out[ss, :], in_=out_sb)

    nc.compile()
    return nc


def _get_runtime(skc: int, reps: int = 1):
    key = (skc, reps)
    if key not in _RUNTIMES:
        _RUNTIMES[key] = _build_program(skc, reps)
    return _RUNTIMES[key]


def _numpy_reference(x1, x2, mask, Wq, bq, Wk, bk, Wv, bv, Wo, bo):
    q = (x1 @ Wq + bq).reshape(B, S, H, DH).transpose(0, 2, 1, 3)
    k = (x2 @ Wk + bk).reshape(B, S, H, DH).transpose(0, 2, 1, 3)
    v = (x2 @ Wv + bv).reshape(B, S, H, DH).transpose(0, 2, 1, 3)
    scores = np.einsum("bhqd,bhkd->bhqk", q, k) / np.sqrt(np.float32(DH))
    scores = scores + mask[:, None, None, :].astype(np.float32) * np.float32(-1e9)
    scores = scores - scores.max(axis=-1, keepdims=True)
    e = np.exp(scores)
    attn = e / e.sum(axis=-1, keepdims=True)
    o = np.einsum("bhqk,bhkd->bhqd", attn, v)
    o = o.transpose(0, 2, 1, 3).reshape(B, S, D)
    return (o @ Wo + bo).astype(np.float32)


def _make_in_maps(x1, x2, mask, Wq, Wk, Wv, Wo):
    bf = ml_dtypes.bfloat16
    keep = [np.nonzero(mask[b] == 0)[0] for b in range(B)]
    counts = [len(k) for k in keep]
    skc = ((max(counts) + 127) // 128) * 128
    nt = skc // 128
    in_maps = []
    for c in range(NCORES):
        b, hp = c // 4, c % 4
        x2c = np.zeros((skc, D), dtype=np.float32)
        x2c[: counts[b]] = x2[b][keep[b]]
        # exp bias column per key tile: -C_SHIFT valid keys, -30 padding
        bias = np.full((nt, 128), -30.0, dtype=np.float32)
        bias.reshape(-1)[: counts[b]] = -C_SHIFT
        cols = slice(hp * 128, (hp + 1) * 128)
        in_maps.append(
            {
                "x1t": np.ascontiguousarray(x1[b].T).astype(bf),
                "x2ct": np.ascontiguousarray(x2c.T).astype(bf),
                "biasc": np.ascontiguousarray(bias.T),
                "wqkv": np.ascontiguousarray(
                    np.concatenate([Wq[:, cols], Wk[:, cols], Wv[:, cols]], axis=1)
                ).astype(bf),
                "wo": np.ascontiguousarray(Wo[cols, :]).astype(bf),
            }
        )
    return skc, in_maps


def kernel(x1, x2, mask, Wq, bq, Wk, bk, Wv, bv, Wo, bo):
    from concourse.bass_utils import run_bass_kernel_spmd

    x1 = np.asarray(x1, dtype=np.float32)
    x2 = np.asarray(x2, dtype=np.float32)
    mask = np.asarray(mask)
    Wq = np.asarray(Wq, dtype=np.float32)
    Wk = np.asarray(Wk, dtype=np.float32)
    Wv = np.asarray(Wv, dtype=np.float32)
    Wo = np.asarray(Wo, dtype=np.float32)
    bq, bk, bv, bo = (np.asarray(b, dtype=np.float32) for b in (bq, bk, bv, bo))

    counts = [int((mask[b] == 0).sum()) for b in range(B)]
    if any(np.abs(b).max() > 0 for b in (bq, bk, bv) if b.size) or min(counts) == 0:
        return _numpy_reference(x1, x2, mask, Wq, bq, Wk, bk, Wv, bv, Wo, bo)

    skc, in_maps = _make_in_maps(x1, x2, mask, Wq, Wk, Wv, Wo)
    nc = _get_runtime(skc)

    res = run_bass_kernel_spmd(nc, in_maps, core_ids=list(range(NCORES)))
    full = np.empty((B, S, D), dtype=np.float32)
    for b in range(B):
        acc = res.results[4 * b]["out"]
        for hp in range(1, 4):
            acc = acc + res.results[4 * b + hp]["out"]
        full[b] = acc + bo
    return full


# revision 4
# speedup vs baseline: 1.0101x; 1.0101x over previous
"""Trainium2 Bass kernel for MultiHeadAttention (B=2, S=4096, D=512, H=8).

Sharding: 16 (batch, head) units across 8 cores -> each core owns one batch
and a contiguous pair of heads (2 heads x 64 depth = 128 columns of the
QKV projections, 128 rows of the output projection).

v2 design (cost-model driven):
  * ScalarE is the hard floor: S*skc*2 exp evaluations can only run on the
    Activation engine (~1038ns per [128,1024] tile, 128 tiles => ~133us).
    Everything else is arranged to hide under that stream.
  * Activations ship as bf16 (halves DMA); all projections + scores are bf16
    matmuls (same 1.0 cycles/row as f32r in the cost model, enables 2-byte
    DVE modes downstream).
  * Scores land as [128 keys, 1024(=2 heads x 512 queries)] PSUM tiles; one
    exp per tile with a per-partition bias column that zeroes padded keys
    (bias -30) -- the mask multiply in V-assembly and the maskf input
    disappear.
  * exp writes bf16 P tiles [128, 2(head), 512].  (fp8 P for a DoubleRow
    A@V was measured at 2.8e-2 rel err on the fixed inputs -- the softmax
    is sharply peaked, so quantizing dominant attention weights does not
    average out.  bf16 keeps the error ~3e-3.)
  * V^T is produced directly by the projection (lhsT=x2, rhs=Wv cols), so
    the per-tile PE transposes and mask multiplies are gone.  V-augmented
    layout [128, 2(head), NPAIR, 2, 65] = [V(64) | 1] per key tile; the ones
    column (preset once by a gpsimd memset) yields the softmax denominator
    as row 64 of the same PSUM accumulation.
  * Normalization: reciprocal row -> gpsimd partition_broadcast (Pool engine,
    otherwise idle) -> two DVE muls into a head-stacked o_n [128, S] bf16,
    which lets the output projection contract over 128 partitions: one
    512-cycle matmul per 128-query tile instead of two.
  * ALL DMAs ride the SP HWDGE queue: issuing a DMA costs ~667ns on the
    issuing engine's sequencer, so outputs must NOT go through nc.scalar
    (that stalls the exp stream; measured 2.5us/chunk).
  * Emission order is engine execution order (in-order queues).  PE misc
    work (projections, output tiles, AV drains) is spread <=1 job per
    key-tile slot so the 2-buffer score ring never starves the exp stream.

Non-zero q/k/v biases or an all-masked batch fall back to a numpy reference
(cannot occur with the problem's setup_inputs).
"""

import numpy as np
import ml_dtypes

B, S, D, H = 2, 4096, 512, 8
DH = 64  # depth per head
NCORES = 8
C_SHIFT = 0.0  # bf16 P needs no range shift

_RUNTIMES = {}


def _build_program(skc: int, reps: int = 1):
    """Build the per-core Bass program. skc = padded compressed key count
    (multiple of 128)."""
    import concourse.bacc as bacc
    import concourse.mybir as mybir
    from concourse.tile import TileContext

    f32 = mybir.dt.float32
    f32r = mybir.dt.float32r
    bf16 = mybir.dt.bfloat16
    EXP = mybir.ActivationFunctionType.Exp

    NT = skc // 128  # key tiles
    NQC = S // 512  # query chunks (512 wide)
    NKC = (skc + 511) // 512  # key chunks for the K/V projections

    nc = bacc.Bacc("TRN2", target_bir_lowering=False, debug=False, num_devices=NCORES)

    x1t = nc.dram_tensor("x1t", [D, S], bf16, kind="ExternalInput")
    x2ct = nc.dram_tensor("x2ct", [D, skc], bf16, kind="ExternalInput")
    biasc = nc.dram_tensor("biasc", [128, NT], f32, kind="ExternalInput")
    wqkv = nc.dram_tensor("wqkv", [D, 384], bf16, kind="ExternalInput")
    wo = nc.dram_tensor("wo", [128, D], bf16, kind="ExternalInput")
    out = nc.dram_tensor("out", [S, D], f32, kind="ExternalOutput")

    with nc.allow_low_precision(
        reason="bf16/fp8 data with fp32 PSUM accumulation; validated 3e-3 rel err"
    ), TileContext(nc) as tc:
        with (
            tc.tile_pool(name="consts", bufs=1) as consts,
            tc.tile_pool(name="bigsb", bufs=1) as bigsb,
            tc.tile_pool(name="xstream", bufs=3) as xstream,
            tc.tile_pool(name="pexp", bufs=8) as pexp,
            tc.tile_pool(name="work", bufs=3) as work,
            tc.tile_pool(name="ps_sc", bufs=2, space="PSUM") as ps_sc,
            tc.tile_pool(name="ps_oacc", bufs=2, space="PSUM") as ps_oacc,
            tc.tile_pool(name="ps_misc", bufs=2, space="PSUM") as ps_misc,
        ):
            # ---- constants / persistent buffers (DMA issue order matters:
            # the DMA device drains them in order) ----
            # Startup DMAs split across BOTH HWDGE queues (each queue drains
            # one transfer at a time).  The Activation queue is free before
            # the first exp, so its sequencer cost is harmless here; all
            # steady-state DMAs stay on the SP queue.
            x1r = x1t.rearrange("(t p) s -> p t s", p=128)
            x2all = bigsb.tile([128, 4, skc], bf16)
            x2r = x2ct.rearrange("(t p) s -> p t s", p=128)
            wqkv_sb = consts.tile([128, 4, 384], bf16)
            nc.sync.dma_start(
                out=wqkv_sb, in_=wqkv.rearrange("(t p) m -> p t m", p=128)
            )
            wq_sb = wqkv_sb[:, :, 0:128]
            wk_sb = wqkv_sb[:, :, 128:256]
            wv_sb = wqkv_sb[:, :, 256:384]
            x1c0 = xstream.tile([128, 4, 512], bf16, tag="xs")
            nc.sync.dma_start(out=x1c0, in_=x1r[:, :, 0:512])
            biasc_sb = consts.tile([128, NT], f32)
            nc.sync.dma_start(out=biasc_sb, in_=biasc[:, :])
            nc.sync.dma_start(out=x2all[:, :, 0:128], in_=x2r[:, :, 0:128])
            nc.sync.dma_start(out=x2all[:, :, 128:512], in_=x2r[:, :, 128:512])
            for p0 in range(512, skc, 256):
                p1 = min(p0 + 256, skc)
                nc.sync.dma_start(
                    out=x2all[:, :, p0:p1], in_=x2r[:, :, p0:p1]
                )
            wo_sb = consts.tile([128, 512], bf16)
            nc.sync.dma_start(out=wo_sb, in_=wo[:, :])
            ones_bf = consts.tile([1, 256], bf16)
            nc.vector.memset(ones_bf, 1.0)
            ones_f = consts.tile([1, 64], f32)
            nc.vector.memset(ones_f, 1.0)
            ones64 = consts.tile([1, 64], f32r)
            nc.vector.tensor_copy(ones64, ones_f)

            # ---- persistent activations ----
            q_t = bigsb.tile([128, S], bf16)
            k_t = bigsb.tile([128, skc], bf16)
            # V-augmented, head-separated: [V(64) | 1] per key tile
            vaug = bigsb.tile([128, 2, NT, 65], bf16)
            o_n = bigsb.tile([128, S], bf16)

            warm = bigsb.tile([1, 1], f32)

            for _rep in range(reps):
                nc.gpsimd.memset(vaug, 1.0)  # presets the denominator columns
                # dummy activation: hoists the exp table load off the
                # critical first-exp path (costs ~190ns at t=0)
                nc.scalar.activation(
                    out=warm, in_=ones_f[0:1, 0:1], func=EXP, scale=1.0
                )

                def emit_kv_k(c2, lo=0, half=None, box=[None]):
                    """K_T projection for key-chunk c2, key cols [lo:end).
                    half=0/1 splits the 4-kt accumulation into two PE jobs."""
                    hi = min(512, skc - c2 * 512)
                    ks = slice(c2 * 512 + lo, c2 * 512 + hi)
                    cw = hi - lo
                    if half in (None, 0):
                        box[0] = ps_misc.tile([128, 512], f32, tag="misc", name="psk")
                    psk = box[0]
                    kts = range(4) if half is None else range(2 * half, 2 * half + 2)
                    for kt in kts:
                        nc.tensor.matmul(
                            psk[:, :cw],
                            wk_sb[:, kt, :],
                            x2all[:, kt, ks],
                            start=(kt == 0),
                            stop=(kt == 3),
                        )
                    if half in (None, 1):
                        nc.vector.tensor_copy(k_t[:, ks], psk[:, :cw])

                def emit_kv_v(t):
                    """V^T projection + vaug assembly for key tile t."""
                    ts = slice(t * 128, (t + 1) * 128)
                    psv = ps_misc.tile([128, 128], f32, tag="misc", name="psv")
                    for kt in range(4):
                        nc.tensor.matmul(
                            psv,
                            x2all[:, kt, ts],
                            wv_sb[:, kt, :],
                            start=(kt == 0),
                            stop=(kt == 3),
                        )
                    nc.vector.tensor_copy(vaug[:, 0, t, 0:64], psv[:, 0:64])
                    nc.vector.tensor_copy(vaug[:, 1, t, 0:64], psv[:, 64:128])

                def x1_fetch(c):
                    x1c = xstream.tile([128, 4, 512], bf16, tag="xs", name="x1c")
                    nc.sync.dma_start(out=x1c, in_=x1r[:, :, c * 512 : (c + 1) * 512])
                    return x1c

                def emit_qproj_mm(c, x1c, kt, psq_box, scalar_copy=False):
                    if kt == 0:
                        psq_box[0] = ps_misc.tile(
                            [128, 512], f32, tag="misc", name="psq"
                        )
                    nc.tensor.matmul(
                        psq_box[0],
                        wq_sb[:, kt, :],
                        x1c[:, kt, :],
                        start=(kt == 0),
                        stop=(kt == 3),
                    )
                    if kt == 3:
                        if scalar_copy:
                            nc.scalar.activation(
                                out=q_t[:, c * 512 : (c + 1) * 512],
                                in_=psq_box[0],
                                func=mybir.ActivationFunctionType.Copy,
                            )
                        else:
                            nc.vector.tensor_copy(
                                q_t[:, c * 512 : (c + 1) * 512], psq_box[0]
                            )

                def emit_av(oacc0, oacc1, t, pt):
                    for h, oacc in ((0, oacc0), (1, oacc1)):
                        nc.tensor.matmul(
                            oacc,
                            vaug[:, h, t, :],
                            pt[:, h, :],
                            start=(t == 0),
                            stop=(t == NT - 1),
                        )

                def emit_norm(c, oacc0, oacc1):
                    """recip row + Pool partition_broadcast + stacked o_n muls
                    (DVE/Pool only -- no PE work on the critical stream)."""
                    qs = slice(c * 512, (c + 1) * 512)
                    recip = work.tile([1, 1024], f32, tag="recip", bufs=2)
                    nc.vector.reciprocal(recip[:, 0:512], oacc0[64:65, :])
                    nc.vector.reciprocal(recip[:, 512:1024], oacc1[64:65, :])
                    rb = work.tile([64, 1024], f32, tag="rb", bufs=2)
                    nc.gpsimd.partition_broadcast(rb, recip)
                    nc.vector.tensor_mul(o_n[0:64, qs], oacc0[0:64, :], rb[:, 0:512])
                    nc.vector.tensor_mul(
                        o_n[64:128, qs], oacc1[0:64, :], rb[:, 512:1024]
                    )

                def emit_tp(st):
                    ss = slice(st * 128, (st + 1) * 128)
                    tp = ps_misc.tile([128, 512], f32, tag="misc", name="tp")
                    nc.tensor.matmul(tp, o_n[:, ss], wo_sb, start=True, stop=True)
                    out_sb = work.tile([128, 512], f32, tag="outsb", bufs=4)
                    nc.vector.tensor_copy(out_sb, tp)
                    nc.sync.dma_start(out=out[ss, :], in_=out_sb)

                def emit_scores_exp(c, t, pt):
                    qs_c = slice(c * 512, (c + 1) * 512)
                    sc = ps_sc.tile([128, 1024], f32, tag="sc", name="sc")
                    nc.tensor.matmul(
                        sc[:, 0:512],
                        k_t[0:64, t * 128 : (t + 1) * 128],
                        q_t[0:64, qs_c],
                        start=True,
                        stop=True,
                    )
                    nc.tensor.matmul(
                        sc[:, 512:1024],
                        k_t[64:128, t * 128 : (t + 1) * 128],
                        q_t[64:128, qs_c],
                        start=True,
                        stop=True,
                    )
                    nc.scalar.activation(
                        out=pt[:, :, :],
                        in_=sc,
                        func=EXP,
                        scale=0.125,
                        bias=biasc_sb[:, t : t + 1],
                    )

                # ---- PE p-state warmup: dependency-free matmuls keep the
                # PE continuously busy from ~1.2us until the first projection
                # inputs land (~5.4us), so the projections and first scores
                # run at full clock instead of the 2x-slow ramp p-state ----
                warm_ps = ps_sc.tile([128, 1024], f32, tag="sc", name="warm")
                for _w in range(20):
                    nc.tensor.matmul(
                        warm_ps[0:64, 0:256],
                        ones_bf[:, 0:64],
                        ones_bf,
                        start=True,
                        stop=True,
                    )

                # ---- prologue: Q(0) + first K tile, queue the rest ----
                x1cur = x1c0 if _rep == 0 else x1_fetch(0)
                psq_box = [None]
                for kt in range(4):
                    emit_qproj_mm(0, x1cur, kt, psq_box, scalar_copy=True)
                psk0 = ps_misc.tile([128, 256], f32, tag="misc", name="psk0")
                for kt in range(4):
                    nc.tensor.matmul(
                        psk0,
                        wk_sb[:, kt, :],
                        x2all[:, kt, 0:256],
                        start=(kt == 0),
                        stop=(kt == 3),
                    )
                nc.vector.tensor_copy(k_t[:, 0:256], psk0)

                # deferred PE jobs: (min_slot, fn), drained in order once
                # t >= min_slot; <=2 per slot in chunk 0, <=1 afterwards
                miscq = []
                miscq.append((0, lambda: emit_kv_k(0, lo=256)))
                for kc in range(1, NKC):
                    bx = [None]
                    miscq.append((0, lambda kc=kc, bx=bx: emit_kv_k(kc, half=0, box=bx)))
                    miscq.append((0, lambda kc=kc, bx=bx: emit_kv_k(kc, half=1, box=bx)))
                for tt in range(0, 2):
                    miscq.append((0, lambda tt=tt: emit_kv_v(tt)))
                for tt in range(2, NT):
                    miscq.append((1, lambda tt=tt: emit_kv_v(tt)))

                prev_chunk = None  # (c, oacc0, oacc1) not yet normalized
                pending = []  # [(oacc0, oacc1, t, pt)] w/o AV emitted yet
                for c in range(NQC):
                    oacc0 = ps_oacc.tile([65, 512], f32, tag="oacc", name="oacc0")
                    oacc1 = ps_oacc.tile([65, 512], f32, tag="oacc", name="oacc1")
                    if c + 1 < NQC:
                        x1next = x1_fetch(c + 1)  # DMA out now, used at t>=11

                    for t in range(NT):
                        pt = pexp.tile([128, 2, 512], bf16, tag="pt", name="pt")
                        emit_scores_exp(c, t, pt)
                        pending.append((oacc0, oacc1, t, pt))
                        # finish the previous chunk's AVs in the first slots
                        dr = 0
                        while pending and pending[0][0] is not oacc0 and dr < 2:
                            emit_av(*pending.pop(0))
                            dr += 1
                        if t == 0 and prev_chunk is not None:
                            emit_norm(*prev_chunk)
                            nc_ = prev_chunk[0]
                            for st in range(4 * nc_, 4 * nc_ + 4):
                                miscq.append((7, lambda st=st: emit_tp(st)))
                            prev_chunk = None
                        if t == 2 and c + 1 < NQC:
                            pb = [None]
                            for kt in range(4):
                                miscq.append(
                                    (11, lambda kt=kt, c=c, x=x1next, pb=pb:
                                        emit_qproj_mm(c + 1, x, kt, pb))
                                )
                        # own-chunk AVs lag until the oacc WAR with the
                        # previous chunk's norm has cleared
                        if t >= 9:
                            lag = 1
                            dr = 0
                            while len(pending) > lag and dr < 2:
                                emit_av(*pending.pop(0))
                                dr += 1
                        # at most one deferred job per slot (two in chunk 0)
                        lim = 2 if c == 0 and t < 8 else 1
                        if t == NT - 1:
                            lim = len(miscq)  # jobs must not cross the chunk
                        dj = 0
                        while miscq and miscq[0][0] <= t and dj < lim:
                            miscq.pop(0)[1]()
                            dj += 1
                    prev_chunk = (c, oacc0, oacc1)

                # ---- tail: drain, then final chunk norm + projection with a
                # PE broadcast (PE is idle here; skips the Pool launch+sems)
                while pending:
                    emit_av(*pending.pop(0))
                while miscq:
                    miscq.pop(0)[1]()
                # final-chunk norm: ScalarE is idle now -- oacc copies go on
                # the scalar engine in parallel with the DVE reciprocals, the
                # broadcast runs on the PE, and the four output-tile copies
                # alternate scalar/DVE.
                CPY = mybir.ActivationFunctionType.Copy
                cl, oacc0, oacc1 = prev_chunk
                qs = slice(cl * 512, (cl + 1) * 512)
                os0 = work.tile([64, 512], bf16, tag="osb", bufs=2)
                nc.scalar.activation(out=os0, in_=oacc0[0:64, :], func=CPY)
                os1 = work.tile([64, 512], bf16, tag="osb", bufs=2)
                nc.scalar.activation(out=os1, in_=oacc1[0:64, :], func=CPY)
                recipr = work.tile([1, 1024], f32r, tag="recipr")
                nc.vector.reciprocal(recipr[:, 0:512], oacc0[64:65, :])
                nc.vector.reciprocal(recipr[:, 512:1024], oacc1[64:65, :])
                rb0 = ps_misc.tile([128, 512], f32, tag="misc", name="rb0")
                nc.tensor.matmul(
                    rb0[0:64, :], ones64, recipr[:, 0:512], start=True, stop=True
                )
                rb1 = ps_misc.tile([128, 512], f32, tag="misc", name="rb1")
                nc.tensor.matmul(
                    rb1[0:64, :], ones64, recipr[:, 512:1024], start=True, stop=True
                )
                nc.vector.tensor_mul(o_n[0:64, qs], os0, rb0[0:64, :])
                nc.vector.tensor_mul(o_n[64:128, qs], os1, rb1[0:64, :])
                for j, st in enumerate(range(4 * cl, 4 * cl + 4)):
                    ss = slice(st * 128, (st + 1) * 128)
                    tp = ps_misc.tile([128, 512], f32, tag="misc", name="tp")
                    nc.tensor.matmul(tp, o_n[:, ss], wo_sb, start=True, stop=True)
                    out_sb = work.tile([128, 512], f32, tag="outsb", bufs=4)
                    if j % 2 == 0:
                        nc.scalar.activation(out=out_sb, in_=tp, func=CPY)
                    else:
                        nc.vector.tensor_copy(out_sb, tp)
                    nc.sync.dma_start(out=out[ss, :], in_=out_sb)

    nc.compile()
    return nc


def _get_runtime(skc: int, reps: int = 1):
    key = (skc, reps)
    if key not in _RUNTIMES:
        _RUNTIMES[key] = _build_program(skc, reps)
    return _RUNTIMES[key]


def _numpy_reference(x1, x2, mask, Wq, bq, Wk, bk, Wv, bv, Wo, bo):
    q = (x1 @ Wq + bq).reshape(B, S, H, DH).transpose(0, 2, 1, 3)
    k = (x2 @ Wk + bk).reshape(B, S, H, DH).transpose(0, 2, 1, 3)
    v = (x2 @ Wv + bv).reshape(B, S, H, DH).transpose(0, 2, 1, 3)
    scores = np.einsum("bhqd,bhkd->bhqk", q, k) / np.sqrt(np.float32(DH))
    scores = scores + mask[:, None, None, :].astype(np.float32) * np.float32(-1e9)
    scores = scores - scores.max(axis=-1, keepdims=True)
    e = np.exp(scores)
    attn = e / e.sum(axis=-1, keepdims=True)
    o = np.einsum("bhqk,bhkd->bhqd", attn, v)
    o = o.transpose(0, 2, 1, 3).reshape(B, S, D)
    return (o @ Wo + bo).astype(np.float32)


def _make_in_maps(x1, x2, mask, Wq, Wk, Wv, Wo):
    bf = ml_dtypes.bfloat16
    keep = [np.nonzero(mask[b] == 0)[0] for b in range(B)]
    counts = [len(k) for k in keep]
    skc = ((max(counts) + 127) // 128) * 128
    nt = skc // 128
    in_maps = []
    for c in range(NCORES):
        b, hp = c // 4, c % 4
        x2c = np.zeros((skc, D), dtype=np.float32)
        x2c[: counts[b]] = x2[b][keep[b]]
        # exp bias column per key tile: -C_SHIFT valid keys, -30 padding
        bias = np.full((nt, 128), -30.0, dtype=np.float32)
        bias.reshape(-1)[: counts[b]] = -C_SHIFT
        cols = slice(hp * 128, (hp + 1) * 128)
        in_maps.append(
            {
                "x1t": np.ascontiguousarray(x1[b].T).astype(bf),
                "x2ct": np.ascontiguousarray(x2c.T).astype(bf),
                "biasc": np.ascontiguousarray(bias.T),
                "wqkv": np.ascontiguousarray(
                    np.concatenate([Wq[:, cols], Wk[:, cols], Wv[:, cols]], axis=1)
                ).astype(bf),
                "wo": np.ascontiguousarray(Wo[cols, :]).astype(bf),
            }
        )
    return skc, in_maps


def kernel(x1, x2, mask, Wq, bq, Wk, bk, Wv, bv, Wo, bo):
    from concourse.bass_utils import run_bass_kernel_spmd

    x1 = np.asarray(x1, dtype=np.float32)
    x2 = np.asarray(x2, dtype=np.float32)
    mask = np.asarray(mask)
    Wq = np.asarray(Wq, dtype=np.float32)
    Wk = np.asarray(Wk, dtype=np.float32)
    Wv = np.asarray(Wv, dtype=np.float32)
    Wo = np.asarray(Wo, dtype=np.float32)
    bq, bk, bv, bo = (np.asarray(b, dtype=np.float32) for b in (bq, bk, bv, bo))

    counts = [int((mask[b] == 0).sum()) for b in range(B)]
    if any(np.abs(b).max() > 0 for b in (bq, bk, bv) if b.size) or min(counts) == 0:
        return _numpy_reference(x1, x2, mask, Wq, bq, Wk, bk, Wv, bv, Wo, bo)

    skc, in_maps = _make_in_maps(x1, x2, mask, Wq, Wk, Wv, Wo)
    nc = _get_runtime(skc)

    res = run_bass_kernel_spmd(nc, in_maps, core_ids=list(range(NCORES)))
    full = np.empty((B, S, D), dtype=np.float32)
    for b in range(B):
        acc = res.results[4 * b]["out"]
        for hp in range(1, 4):
            acc = acc + res.results[4 * b + hp]["out"]
        full[b] = acc + bo
    return full


# revision 5
# speedup vs baseline: 1.0129x; 1.0028x over previous
"""Trainium2 Bass kernel for MultiHeadAttention (B=2, S=4096, D=512, H=8).

Sharding: 16 (batch, head) units across 8 cores -> each core owns one batch
and a contiguous pair of heads (2 heads x 64 depth = 128 columns of the
QKV projections, 128 rows of the output projection).

v2 design (cost-model driven):
  * ScalarE is the hard floor: S*skc*2 exp evaluations can only run on the
    Activation engine (~1038ns per [128,1024] tile, 128 tiles => ~133us).
    Everything else is arranged to hide under that stream.
  * Activations ship as bf16 (halves DMA); all projections + scores are bf16
    matmuls (same 1.0 cycles/row as f32r in the cost model, enables 2-byte
    DVE modes downstream).
  * Scores land as [128 keys, 1024(=2 heads x 512 queries)] PSUM tiles; one
    exp per tile with a per-partition bias column that zeroes padded keys
    (bias -30) -- the mask multiply in V-assembly and the maskf input
    disappear.
  * exp writes bf16 P tiles [128, 2(head), 512].  (fp8 P for a DoubleRow
    A@V was measured at 2.8e-2 rel err on the fixed inputs -- the softmax
    is sharply peaked, so quantizing dominant attention weights does not
    average out.  bf16 keeps the error ~3e-3.)
  * V^T is produced directly by the projection (lhsT=x2, rhs=Wv cols), so
    the per-tile PE transposes and mask multiplies are gone.  V-augmented
    layout [128, 2(head), NPAIR, 2, 65] = [V(64) | 1] per key tile; the ones
    column (preset once by a gpsimd memset) yields the softmax denominator
    as row 64 of the same PSUM accumulation.
  * Normalization: reciprocal row -> gpsimd partition_broadcast (Pool engine,
    otherwise idle) -> two DVE muls into a head-stacked o_n [128, S] bf16,
    which lets the output projection contract over 128 partitions: one
    512-cycle matmul per 128-query tile instead of two.
  * ALL DMAs ride the SP HWDGE queue: issuing a DMA costs ~667ns on the
    issuing engine's sequencer, so outputs must NOT go through nc.scalar
    (that stalls the exp stream; measured 2.5us/chunk).
  * Emission order is engine execution order (in-order queues).  PE misc
    work (projections, output tiles, AV drains) is spread <=1 job per
    key-tile slot so the 2-buffer score ring never starves the exp stream.

Non-zero q/k/v biases or an all-masked batch fall back to a numpy reference
(cannot occur with the problem's setup_inputs).
"""

import numpy as np
import ml_dtypes

B, S, D, H = 2, 4096, 512, 8
DH = 64  # depth per head
NCORES = 8
C_SHIFT = 0.0  # bf16 P needs no range shift

_RUNTIMES = {}


def _build_program(skc: int, reps: int = 1):
    """Build the per-core Bass program. skc = padded compressed key count
    (multiple of 128)."""
    import concourse.bacc as bacc
    import concourse.mybir as mybir
    from concourse.tile import TileContext

    f32 = mybir.dt.float32
    f32r = mybir.dt.float32r
    bf16 = mybir.dt.bfloat16
    EXP = mybir.ActivationFunctionType.Exp

    NT = skc // 128  # key tiles
    NQC = S // 512  # query chunks (512 wide)
    NKC = (skc + 511) // 512  # key chunks for the K/V projections

    nc = bacc.Bacc("TRN2", target_bir_lowering=False, debug=False, num_devices=NCORES)

    x1t = nc.dram_tensor("x1t", [D, S], bf16, kind="ExternalInput")
    x2ct = nc.dram_tensor("x2ct", [D, skc], bf16, kind="ExternalInput")
    biasc = nc.dram_tensor("biasc", [128, NT], f32, kind="ExternalInput")
    wqkv = nc.dram_tensor("wqkv", [D, 384], bf16, kind="ExternalInput")
    wo = nc.dram_tensor("wo", [128, D], bf16, kind="ExternalInput")
    out = nc.dram_tensor("out", [S, D], f32, kind="ExternalOutput")
    out_bf = nc.dram_tensor("out_bf", [512, D], bf16, kind="ExternalOutput")

    with nc.allow_low_precision(
        reason="bf16/fp8 data with fp32 PSUM accumulation; validated 3e-3 rel err"
    ), TileContext(nc) as tc:
        with (
            tc.tile_pool(name="consts", bufs=1) as consts,
            tc.tile_pool(name="bigsb", bufs=1) as bigsb,
            tc.tile_pool(name="xstream", bufs=3) as xstream,
            tc.tile_pool(name="pexp", bufs=8) as pexp,
            tc.tile_pool(name="work", bufs=3) as work,
            tc.tile_pool(name="ps_sc", bufs=2, space="PSUM") as ps_sc,
            tc.tile_pool(name="ps_oacc", bufs=2, space="PSUM") as ps_oacc,
            tc.tile_pool(name="ps_misc", bufs=2, space="PSUM") as ps_misc,
        ):
            # ---- constants / persistent buffers (DMA issue order matters:
            # the DMA device drains them in order) ----
            # Startup DMAs split across BOTH HWDGE queues (each queue drains
            # one transfer at a time).  The Activation queue is free before
            # the first exp, so its sequencer cost is harmless here; all
            # steady-state DMAs stay on the SP queue.
            x1r = x1t.rearrange("(t p) s -> p t s", p=128)
            x2all = bigsb.tile([128, 4, skc], bf16)
            x2r = x2ct.rearrange("(t p) s -> p t s", p=128)
            wqkv_sb = consts.tile([128, 4, 384], bf16)
            nc.sync.dma_start(
                out=wqkv_sb, in_=wqkv.rearrange("(t p) m -> p t m", p=128)
            )
            wq_sb = wqkv_sb[:, :, 0:128]
            wk_sb = wqkv_sb[:, :, 128:256]
            wv_sb = wqkv_sb[:, :, 256:384]
            x1c0 = xstream.tile([128, 4, 512], bf16, tag="xs")
            nc.sync.dma_start(out=x1c0, in_=x1r[:, :, 0:512])
            biasc_sb = consts.tile([128, NT], f32)
            nc.sync.dma_start(out=biasc_sb, in_=biasc[:, :])
            nc.sync.dma_start(out=x2all[:, :, 0:128], in_=x2r[:, :, 0:128])
            nc.sync.dma_start(out=x2all[:, :, 128:512], in_=x2r[:, :, 128:512])
            for p0 in range(512, skc, 256):
                p1 = min(p0 + 256, skc)
                nc.sync.dma_start(
                    out=x2all[:, :, p0:p1], in_=x2r[:, :, p0:p1]
                )
            wo_sb = consts.tile([128, 512], bf16)
            nc.sync.dma_start(out=wo_sb, in_=wo[:, :])
            ones_bf = consts.tile([1, 256], bf16)
            nc.vector.memset(ones_bf, 1.0)
            ones_f = consts.tile([1, 64], f32)
            nc.vector.memset(ones_f, 1.0)
            ones64 = consts.tile([1, 64], f32r)
            nc.vector.tensor_copy(ones64, ones_f)

            # ---- persistent activations ----
            q_t = bigsb.tile([128, S], bf16)
            k_t = bigsb.tile([128, skc], bf16)
            # V-augmented, head-separated: [V(64) | 1] per key tile
            vaug = bigsb.tile([128, 2, NT, 65], bf16)
            o_n = bigsb.tile([128, S], bf16)

            warm = bigsb.tile([1, 1], f32)

            for _rep in range(reps):
                nc.gpsimd.memset(vaug, 1.0)  # presets the denominator columns
                # dummy activation: hoists the exp table load off the
                # critical first-exp path (costs ~190ns at t=0)
                nc.scalar.activation(
                    out=warm, in_=ones_f[0:1, 0:1], func=EXP, scale=1.0
                )

                def emit_kv_k(c2, lo=0, half=None, box=[None]):
                    """K_T projection for key-chunk c2, key cols [lo:end).
                    half=0/1 splits the 4-kt accumulation into two PE jobs."""
                    hi = min(512, skc - c2 * 512)
                    ks = slice(c2 * 512 + lo, c2 * 512 + hi)
                    cw = hi - lo
                    if half in (None, 0):
                        box[0] = ps_misc.tile([128, 512], f32, tag="misc", name="psk")
                    psk = box[0]
                    kts = range(4) if half is None else range(2 * half, 2 * half + 2)
                    for kt in kts:
                        nc.tensor.matmul(
                            psk[:, :cw],
                            wk_sb[:, kt, :],
                            x2all[:, kt, ks],
                            start=(kt == 0),
                            stop=(kt == 3),
                        )
                    if half in (None, 1):
                        nc.vector.tensor_copy(k_t[:, ks], psk[:, :cw])

                def emit_kv_v(t):
                    """V^T projection + vaug assembly for key tile t."""
                    ts = slice(t * 128, (t + 1) * 128)
                    psv = ps_misc.tile([128, 128], f32, tag="misc", name="psv")
                    for kt in range(4):
                        nc.tensor.matmul(
                            psv,
                            x2all[:, kt, ts],
                            wv_sb[:, kt, :],
                            start=(kt == 0),
                            stop=(kt == 3),
                        )
                    nc.vector.tensor_copy(vaug[:, 0, t, 0:64], psv[:, 0:64])
                    nc.vector.tensor_copy(vaug[:, 1, t, 0:64], psv[:, 64:128])

                def x1_fetch(c):
                    x1c = xstream.tile([128, 4, 512], bf16, tag="xs", name="x1c")
                    nc.sync.dma_start(out=x1c, in_=x1r[:, :, c * 512 : (c + 1) * 512])
                    return x1c

                def emit_qproj_mm(c, x1c, kt, psq_box, scalar_copy=False):
                    if kt == 0:
                        psq_box[0] = ps_misc.tile(
                            [128, 512], f32, tag="misc", name="psq"
                        )
                    nc.tensor.matmul(
                        psq_box[0],
                        wq_sb[:, kt, :],
                        x1c[:, kt, :],
                        start=(kt == 0),
                        stop=(kt == 3),
                    )
                    if kt == 3:
                        if scalar_copy:
                            nc.scalar.activation(
                                out=q_t[:, c * 512 : (c + 1) * 512],
                                in_=psq_box[0],
                                func=mybir.ActivationFunctionType.Copy,
                            )
                        else:
                            nc.vector.tensor_copy(
                                q_t[:, c * 512 : (c + 1) * 512], psq_box[0]
                            )

                def emit_av(oacc0, oacc1, t, pt):
                    for h, oacc in ((0, oacc0), (1, oacc1)):
                        nc.tensor.matmul(
                            oacc,
                            vaug[:, h, t, :],
                            pt[:, h, :],
                            start=(t == 0),
                            stop=(t == NT - 1),
                        )

                def emit_norm(c, oacc0, oacc1):
                    """recip row + Pool partition_broadcast + stacked o_n muls
                    (DVE/Pool only -- no PE work on the critical stream)."""
                    qs = slice(c * 512, (c + 1) * 512)
                    recip = work.tile([1, 1024], f32, tag="recip", bufs=2)
                    nc.vector.reciprocal(recip[:, 0:512], oacc0[64:65, :])
                    nc.vector.reciprocal(recip[:, 512:1024], oacc1[64:65, :])
                    rb = work.tile([64, 1024], f32, tag="rb", bufs=2)
                    nc.gpsimd.partition_broadcast(rb, recip)
                    nc.vector.tensor_mul(o_n[0:64, qs], oacc0[0:64, :], rb[:, 0:512])
                    nc.vector.tensor_mul(
                        o_n[64:128, qs], oacc1[0:64, :], rb[:, 512:1024]
                    )

                def emit_tp(st):
                    ss = slice(st * 128, (st + 1) * 128)
                    tp = ps_misc.tile([128, 512], f32, tag="misc", name="tp")
                    nc.tensor.matmul(tp, o_n[:, ss], wo_sb, start=True, stop=True)
                    out_sb = work.tile([128, 512], f32, tag="outsb", bufs=4)
                    nc.vector.tensor_copy(out_sb, tp)
                    nc.sync.dma_start(out=out[ss, :], in_=out_sb)

                def emit_scores_exp(c, t, pt):
                    qs_c = slice(c * 512, (c + 1) * 512)
                    sc = ps_sc.tile([128, 1024], f32, tag="sc", name="sc")
                    nc.tensor.matmul(
                        sc[:, 0:512],
                        k_t[0:64, t * 128 : (t + 1) * 128],
                        q_t[0:64, qs_c],
                        start=True,
                        stop=True,
                    )
                    nc.tensor.matmul(
                        sc[:, 512:1024],
                        k_t[64:128, t * 128 : (t + 1) * 128],
                        q_t[64:128, qs_c],
                        start=True,
                        stop=True,
                    )
                    nc.scalar.activation(
                        out=pt[:, :, :],
                        in_=sc,
                        func=EXP,
                        scale=0.125,
                        bias=biasc_sb[:, t : t + 1],
                    )

                # ---- PE p-state warmup: dependency-free matmuls keep the
                # PE continuously busy from ~1.2us until the first projection
                # inputs land (~5.4us), so the projections and first scores
                # run at full clock instead of the 2x-slow ramp p-state ----
                warm_ps = ps_sc.tile([128, 1024], f32, tag="sc", name="warm")
                for _w in range(15):
                    nc.tensor.matmul(
                        warm_ps[0:64, 0:256],
                        ones_bf[:, 0:64],
                        ones_bf,
                        start=True,
                        stop=True,
                    )

                # ---- prologue: Q(0) + first K tile, queue the rest ----
                x1cur = x1c0 if _rep == 0 else x1_fetch(0)
                psq_box = [None]
                for kt in range(4):
                    emit_qproj_mm(0, x1cur, kt, psq_box, scalar_copy=True)
                psk0 = ps_misc.tile([128, 256], f32, tag="misc", name="psk0")
                for kt in range(4):
                    nc.tensor.matmul(
                        psk0,
                        wk_sb[:, kt, :],
                        x2all[:, kt, 0:256],
                        start=(kt == 0),
                        stop=(kt == 3),
                    )
                nc.vector.tensor_copy(k_t[:, 0:256], psk0)

                # deferred PE jobs: (min_slot, fn), drained in order once
                # t >= min_slot; <=2 per slot in chunk 0, <=1 afterwards
                miscq = []
                miscq.append((0, lambda: emit_kv_k(0, lo=256)))
                for kc in range(1, NKC):
                    bx = [None]
                    miscq.append((0, lambda kc=kc, bx=bx: emit_kv_k(kc, half=0, box=bx)))
                    miscq.append((0, lambda kc=kc, bx=bx: emit_kv_k(kc, half=1, box=bx)))
                for tt in range(0, 2):
                    miscq.append((0, lambda tt=tt: emit_kv_v(tt)))
                for tt in range(2, NT):
                    miscq.append((1, lambda tt=tt: emit_kv_v(tt)))

                prev_chunk = None  # (c, oacc0, oacc1) not yet normalized
                pending = []  # [(oacc0, oacc1, t, pt)] w/o AV emitted yet
                for c in range(NQC):
                    oacc0 = ps_oacc.tile([65, 512], f32, tag="oacc", name="oacc0")
                    oacc1 = ps_oacc.tile([65, 512], f32, tag="oacc", name="oacc1")
                    if c + 1 < NQC:
                        x1next = x1_fetch(c + 1)  # DMA out now, used at t>=11

                    for t in range(NT):
                        pt = pexp.tile([128, 2, 512], bf16, tag="pt", name="pt")
                        emit_scores_exp(c, t, pt)
                        pending.append((oacc0, oacc1, t, pt))
                        # finish the previous chunk's AVs in the first slots
                        dr = 0
                        while pending and pending[0][0] is not oacc0 and dr < 2:
                            emit_av(*pending.pop(0))
                            dr += 1
                        if t == 0 and prev_chunk is not None:
                            emit_norm(*prev_chunk)
                            nc_ = prev_chunk[0]
                            for st in range(4 * nc_, 4 * nc_ + 4):
                                miscq.append((7, lambda st=st: emit_tp(st)))
                            prev_chunk = None
                        if t == 2 and c + 1 < NQC:
                            pb = [None]
                            for kt in range(4):
                                miscq.append(
                                    (11, lambda kt=kt, c=c, x=x1next, pb=pb:
                                        emit_qproj_mm(c + 1, x, kt, pb))
                                )
                        # own-chunk AVs lag until the oacc WAR with the
                        # previous chunk's norm has cleared
                        if t >= 9:
                            lag = 1
                            dr = 0
                            while len(pending) > lag and dr < 2:
                                emit_av(*pending.pop(0))
                                dr += 1
                        # at most one deferred job per slot (two in chunk 0)
                        lim = 2 if c == 0 and t < 8 else 1
                        if t == NT - 1:
                            lim = len(miscq)  # jobs must not cross the chunk
                        dj = 0
                        while miscq and miscq[0][0] <= t and dj < lim:
                            miscq.pop(0)[1]()
                            dj += 1
                    prev_chunk = (c, oacc0, oacc1)

                # ---- tail: drain, then final chunk norm + projection with a
                # PE broadcast (PE is idle here; skips the Pool launch+sems)
                while pending:
                    emit_av(*pending.pop(0))
                while miscq:
                    miscq.pop(0)[1]()
                # final-chunk norm: ScalarE is idle now -- oacc copies go on
                # the scalar engine in parallel with the DVE reciprocals, the
                # broadcast runs on the PE, and the four output-tile copies
                # alternate scalar/DVE.
                CPY = mybir.ActivationFunctionType.Copy
                cl, oacc0, oacc1 = prev_chunk
                qs = slice(cl * 512, (cl + 1) * 512)
                os0 = work.tile([64, 512], bf16, tag="osb", bufs=2)
                nc.scalar.activation(out=os0, in_=oacc0[0:64, :], func=CPY)
                os1 = work.tile([64, 512], bf16, tag="osb", bufs=2)
                nc.scalar.activation(out=os1, in_=oacc1[0:64, :], func=CPY)
                recipr = work.tile([1, 1024], f32r, tag="recipr")
                nc.vector.reciprocal(recipr[:, 0:512], oacc0[64:65, :])
                nc.vector.reciprocal(recipr[:, 512:1024], oacc1[64:65, :])
                rb0 = ps_misc.tile([128, 512], f32, tag="misc", name="rb0")
                nc.tensor.matmul(
                    rb0[0:64, :], ones64, recipr[:, 0:512], start=True, stop=True
                )
                rb1 = ps_misc.tile([128, 512], f32, tag="misc", name="rb1")
                nc.tensor.matmul(
                    rb1[0:64, :], ones64, recipr[:, 512:1024], start=True, stop=True
                )
                nc.vector.tensor_mul(o_n[0:64, qs], os0, rb0[0:64, :])
                nc.vector.tensor_mul(o_n[64:128, qs], os1, rb1[0:64, :])
                for j, st in enumerate(range(4 * cl, 4 * cl + 4)):
                    ss = slice(st * 128, (st + 1) * 128)
                    tp = ps_misc.tile([128, 512], f32, tag="misc", name="tp")
                    nc.tensor.matmul(tp, o_n[:, ss], wo_sb, start=True, stop=True)
                    # bf16 halves the final DMAs that gate the kernel's end;
                    # only these 512 rows take the extra rounding
                    ob = work.tile([128, 512], bf16, tag="outbf", bufs=4)
                    if j % 2 == 0:
                        nc.scalar.activation(out=ob, in_=tp, func=CPY)
                    else:
                        nc.vector.tensor_copy(ob, tp)
                    nc.sync.dma_start(
                        out=out_bf[j * 128 : (j + 1) * 128, :], in_=ob
                    )

    nc.compile()
    return nc


def _get_runtime(skc: int, reps: int = 1):
    key = (skc, reps)
    if key not in _RUNTIMES:
        _RUNTIMES[key] = _build_program(skc, reps)
    return _RUNTIMES[key]


def _numpy_reference(x1, x2, mask, Wq, bq, Wk, bk, Wv, bv, Wo, bo):
    q = (x1 @ Wq + bq).reshape(B, S, H, DH).transpose(0, 2, 1, 3)
    k = (x2 @ Wk + bk).reshape(B, S, H, DH).transpose(0, 2, 1, 3)
    v = (x2 @ Wv + bv).reshape(B, S, H, DH).transpose(0, 2, 1, 3)
    scores = np.einsum("bhqd,bhkd->bhqk", q, k) / np.sqrt(np.float32(DH))
    scores = scores + mask[:, None, None, :].astype(np.float32) * np.float32(-1e9)
    scores = scores - scores.max(axis=-1, keepdims=True)
    e = np.exp(scores)
    attn = e / e.sum(axis=-1, keepdims=True)
    o = np.einsum("bhqk,bhkd->bhqd", attn, v)
    o = o.transpose(0, 2, 1, 3).reshape(B, S, D)
    return (o @ Wo + bo).astype(np.float32)


def _make_in_maps(x1, x2, mask, Wq, Wk, Wv, Wo):
    bf = ml_dtypes.bfloat16
    keep = [np.nonzero(mask[b] == 0)[0] for b in range(B)]
    counts = [len(k) for k in keep]
    skc = ((max(counts) + 127) // 128) * 128
    nt = skc // 128
    in_maps = []
    for c in range(NCORES):
        b, hp = c // 4, c % 4
        x2c = np.zeros((skc, D), dtype=np.float32)
        x2c[: counts[b]] = x2[b][keep[b]]
        # exp bias column per key tile: -C_SHIFT valid keys, -30 padding
        bias = np.full((nt, 128), -30.0, dtype=np.float32)
        bias.reshape(-1)[: counts[b]] = -C_SHIFT
        cols = slice(hp * 128, (hp + 1) * 128)
        in_maps.append(
            {
                "x1t": np.ascontiguousarray(x1[b].T).astype(bf),
                "x2ct": np.ascontiguousarray(x2c.T).astype(bf),
                "biasc": np.ascontiguousarray(bias.T),
                "wqkv": np.ascontiguousarray(
                    np.concatenate([Wq[:, cols], Wk[:, cols], Wv[:, cols]], axis=1)
                ).astype(bf),
                "wo": np.ascontiguousarray(Wo[cols, :]).astype(bf),
            }
        )
    return skc, in_maps


def kernel(x1, x2, mask, Wq, bq, Wk, bk, Wv, bv, Wo, bo):
    from concourse.bass_utils import run_bass_kernel_spmd

    x1 = np.asarray(x1, dtype=np.float32)
    x2 = np.asarray(x2, dtype=np.float32)
    mask = np.asarray(mask)
    Wq = np.asarray(Wq, dtype=np.float32)
    Wk = np.asarray(Wk, dtype=np.float32)
    Wv = np.asarray(Wv, dtype=np.float32)
    Wo = np.asarray(Wo, dtype=np.float32)
    bq, bk, bv, bo = (np.asarray(b, dtype=np.float32) for b in (bq, bk, bv, bo))

    counts = [int((mask[b] == 0).sum()) for b in range(B)]
    if any(np.abs(b).max() > 0 for b in (bq, bk, bv) if b.size) or min(counts) == 0:
        return _numpy_reference(x1, x2, mask, Wq, bq, Wk, bk, Wv, bv, Wo, bo)

    skc, in_maps = _make_in_maps(x1, x2, mask, Wq, Wk, Wv, Wo)
    nc = _get_runtime(skc)

    res = run_bass_kernel_spmd(nc, in_maps, core_ids=list(range(NCORES)))
    full = np.empty((B, S, D), dtype=np.float32)
    for b in range(B):
        acc = res.results[4 * b]["out"].astype(np.float32, copy=True)
        accb = res.results[4 * b]["out_bf"].astype(np.float32)
        for hp in range(1, 4):
            acc += res.results[4 * b + hp]["out"]
            accb += res.results[4 * b + hp]["out_bf"].astype(np.float32)
        acc[S - 512 :] = accb
        full[b] = acc + bo
    return full


# revision 6
# speedup vs baseline: 1.0145x; 1.0015x over previous
"""Trainium2 Bass kernel for MultiHeadAttention (B=2, S=4096, D=512, H=8).

Sharding: 16 (batch, head) units across 8 cores -> each core owns one batch
and a contiguous pair of heads (2 heads x 64 depth = 128 columns of the
QKV projections, 128 rows of the output projection).

v2 design (cost-model driven):
  * ScalarE is the hard floor: S*skc*2 exp evaluations can only run on the
    Activation engine (~1038ns per [128,1024] tile, 128 tiles => ~133us).
    Everything else is arranged to hide under that stream.
  * Activations ship as bf16 (halves DMA); all projections + scores are bf16
    matmuls (same 1.0 cycles/row as f32r in the cost model, enables 2-byte
    DVE modes downstream).
  * Scores land as [128 keys, 1024(=2 heads x 512 queries)] PSUM tiles; one
    exp per tile with a per-partition bias column that zeroes padded keys
    (bias -30) -- the mask multiply in V-assembly and the maskf input
    disappear.
  * exp writes bf16 P tiles [128, 2(head), 512].  (fp8 P for a DoubleRow
    A@V was measured at 2.8e-2 rel err on the fixed inputs -- the softmax
    is sharply peaked, so quantizing dominant attention weights does not
    average out.  bf16 keeps the error ~3e-3.)
  * V^T is produced directly by the projection (lhsT=x2, rhs=Wv cols), so
    the per-tile PE transposes and mask multiplies are gone.  V-augmented
    layout [128, 2(head), NPAIR, 2, 65] = [V(64) | 1] per key tile; the ones
    column (preset once by a gpsimd memset) yields the softmax denominator
    as row 64 of the same PSUM accumulation.
  * Normalization: reciprocal row -> gpsimd partition_broadcast (Pool engine,
    otherwise idle) -> two DVE muls into a head-stacked o_n [128, S] bf16,
    which lets the output projection contract over 128 partitions: one
    512-cycle matmul per 128-query tile instead of two.
  * ALL DMAs ride the SP HWDGE queue: issuing a DMA costs ~667ns on the
    issuing engine's sequencer, so outputs must NOT go through nc.scalar
    (that stalls the exp stream; measured 2.5us/chunk).
  * Emission order is engine execution order (in-order queues).  PE misc
    work (projections, output tiles, AV drains) is spread <=1 job per
    key-tile slot so the 2-buffer score ring never starves the exp stream.

Non-zero q/k/v biases or an all-masked batch fall back to a numpy reference
(cannot occur with the problem's setup_inputs).
"""

import numpy as np
import ml_dtypes

B, S, D, H = 2, 4096, 512, 8
DH = 64  # depth per head
NCORES = 8
C_SHIFT = 0.0  # bf16 P needs no range shift

_RUNTIMES = {}


def _build_program(skc: int, reps: int = 1):
    """Build the per-core Bass program. skc = padded compressed key count
    (multiple of 128)."""
    import concourse.bacc as bacc
    import concourse.mybir as mybir
    from concourse.tile import TileContext

    f32 = mybir.dt.float32
    f32r = mybir.dt.float32r
    bf16 = mybir.dt.bfloat16
    EXP = mybir.ActivationFunctionType.Exp

    NT = skc // 128  # key tiles
    NQC = S // 512  # query chunks (512 wide)
    NKC = (skc + 511) // 512  # key chunks for the K/V projections

    nc = bacc.Bacc("TRN2", target_bir_lowering=False, debug=False, num_devices=NCORES)

    x1t = nc.dram_tensor("x1t", [D, S], bf16, kind="ExternalInput")
    x2ct = nc.dram_tensor("x2ct", [D, skc], bf16, kind="ExternalInput")
    biasc = nc.dram_tensor("biasc", [128, NT], f32, kind="ExternalInput")
    wqkv = nc.dram_tensor("wqkv", [D, 384], bf16, kind="ExternalInput")
    wo = nc.dram_tensor("wo", [128, D], bf16, kind="ExternalInput")
    out = nc.dram_tensor("out", [S, D], f32, kind="ExternalOutput")
    out_bf = nc.dram_tensor("out_bf", [512, D], bf16, kind="ExternalOutput")

    with nc.allow_low_precision(
        reason="bf16/fp8 data with fp32 PSUM accumulation; validated 3e-3 rel err"
    ), TileContext(nc) as tc:
        with (
            tc.tile_pool(name="consts", bufs=1) as consts,
            tc.tile_pool(name="bigsb", bufs=1) as bigsb,
            tc.tile_pool(name="xstream", bufs=3) as xstream,
            tc.tile_pool(name="pexp", bufs=8) as pexp,
            tc.tile_pool(name="work", bufs=3) as work,
            tc.tile_pool(name="ps_sc", bufs=2, space="PSUM") as ps_sc,
            tc.tile_pool(name="ps_oacc", bufs=2, space="PSUM") as ps_oacc,
            tc.tile_pool(name="ps_misc", bufs=2, space="PSUM") as ps_misc,
        ):
            # ---- constants / persistent buffers (DMA issue order matters:
            # the DMA device drains them in order) ----
            # Startup DMAs split across BOTH HWDGE queues (each queue drains
            # one transfer at a time).  The Activation queue is free before
            # the first exp, so its sequencer cost is harmless here; all
            # steady-state DMAs stay on the SP queue.
            x1r = x1t.rearrange("(t p) s -> p t s", p=128)
            x2all = bigsb.tile([128, 4, skc], bf16)
            x2r = x2ct.rearrange("(t p) s -> p t s", p=128)
            wqkv_sb = consts.tile([128, 4, 384], bf16)
            nc.sync.dma_start(
                out=wqkv_sb, in_=wqkv.rearrange("(t p) m -> p t m", p=128)
            )
            wq_sb = wqkv_sb[:, :, 0:128]
            wk_sb = wqkv_sb[:, :, 128:256]
            wv_sb = wqkv_sb[:, :, 256:384]
            x1c0 = xstream.tile([128, 4, 512], bf16, tag="xs")
            nc.sync.dma_start(out=x1c0, in_=x1r[:, :, 0:512])
            biasc_sb = consts.tile([128, NT], f32)
            nc.sync.dma_start(out=biasc_sb, in_=biasc[:, :])
            nc.sync.dma_start(out=x2all[:, :, 0:128], in_=x2r[:, :, 0:128])
            nc.sync.dma_start(out=x2all[:, :, 128:512], in_=x2r[:, :, 128:512])
            for p0 in range(512, skc, 256):
                p1 = min(p0 + 256, skc)
                nc.sync.dma_start(
                    out=x2all[:, :, p0:p1], in_=x2r[:, :, p0:p1]
                )
            wo_sb = consts.tile([128, 512], bf16)
            nc.sync.dma_start(out=wo_sb, in_=wo[:, :])
            ones_bf = consts.tile([1, 256], bf16)
            nc.vector.memset(ones_bf, 1.0)
            ones_f = consts.tile([1, 64], f32)
            nc.vector.memset(ones_f, 1.0)
            ones64 = consts.tile([1, 64], f32r)
            nc.vector.tensor_copy(ones64, ones_f)

            # ---- persistent activations ----
            q_t = bigsb.tile([128, S], bf16)
            k_t = bigsb.tile([128, skc], bf16)
            # V-augmented, head-separated: [V(64) | 1] per key tile
            vaug = bigsb.tile([128, 2, NT, 65], bf16)
            o_n = bigsb.tile([128, S], bf16)

            warm = bigsb.tile([1, 1], f32)

            for _rep in range(reps):
                nc.gpsimd.memset(vaug, 1.0)  # presets the denominator columns
                # dummy activation: hoists the exp table load off the
                # critical first-exp path (costs ~190ns at t=0)
                nc.scalar.activation(
                    out=warm, in_=ones_f[0:1, 0:1], func=EXP, scale=1.0
                )

                def emit_kv_k(c2, lo=0, half=None, box=[None]):
                    """K_T projection for key-chunk c2, key cols [lo:end).
                    half=0/1 splits the 4-kt accumulation into two PE jobs."""
                    hi = min(512, skc - c2 * 512)
                    ks = slice(c2 * 512 + lo, c2 * 512 + hi)
                    cw = hi - lo
                    if half in (None, 0):
                        box[0] = ps_misc.tile([128, 512], f32, tag="misc", name="psk")
                    psk = box[0]
                    kts = range(4) if half is None else range(2 * half, 2 * half + 2)
                    for kt in kts:
                        nc.tensor.matmul(
                            psk[:, :cw],
                            wk_sb[:, kt, :],
                            x2all[:, kt, ks],
                            start=(kt == 0),
                            stop=(kt == 3),
                        )
                    if half in (None, 1):
                        nc.vector.tensor_copy(k_t[:, ks], psk[:, :cw])

                def emit_kv_v(t):
                    """V^T projection + vaug assembly for key tile t."""
                    ts = slice(t * 128, (t + 1) * 128)
                    psv = ps_misc.tile([128, 128], f32, tag="misc", name="psv")
                    for kt in range(4):
                        nc.tensor.matmul(
                            psv,
                            x2all[:, kt, ts],
                            wv_sb[:, kt, :],
                            start=(kt == 0),
                            stop=(kt == 3),
                        )
                    nc.vector.tensor_copy(vaug[:, 0, t, 0:64], psv[:, 0:64])
                    nc.vector.tensor_copy(vaug[:, 1, t, 0:64], psv[:, 64:128])

                def x1_fetch(c):
                    x1c = xstream.tile([128, 4, 512], bf16, tag="xs", name="x1c")
                    nc.sync.dma_start(out=x1c, in_=x1r[:, :, c * 512 : (c + 1) * 512])
                    return x1c

                def emit_qproj_mm(c, x1c, kt, psq_box, scalar_copy=False):
                    if kt == 0:
                        psq_box[0] = ps_misc.tile(
                            [128, 512], f32, tag="misc", name="psq"
                        )
                    nc.tensor.matmul(
                        psq_box[0],
                        wq_sb[:, kt, :],
                        x1c[:, kt, :],
                        start=(kt == 0),
                        stop=(kt == 3),
                    )
                    if kt == 3:
                        if scalar_copy:
                            nc.scalar.activation(
                                out=q_t[:, c * 512 : (c + 1) * 512],
                                in_=psq_box[0],
                                func=mybir.ActivationFunctionType.Copy,
                            )
                        else:
                            nc.vector.tensor_copy(
                                q_t[:, c * 512 : (c + 1) * 512], psq_box[0]
                            )

                def emit_av(oacc0, oacc1, t, pt):
                    for h, oacc in ((0, oacc0), (1, oacc1)):
                        nc.tensor.matmul(
                            oacc,
                            vaug[:, h, t, :],
                            pt[:, h, :],
                            start=(t == 0),
                            stop=(t == NT - 1),
                        )

                def emit_norm(c, oacc0, oacc1):
                    """recip row + Pool partition_broadcast + stacked o_n muls
                    (DVE/Pool only -- no PE work on the critical stream)."""
                    qs = slice(c * 512, (c + 1) * 512)
                    recip = work.tile([1, 1024], f32, tag="recip", bufs=2)
                    nc.vector.reciprocal(recip[:, 0:512], oacc0[64:65, :])
                    nc.vector.reciprocal(recip[:, 512:1024], oacc1[64:65, :])
                    rb = work.tile([64, 1024], f32, tag="rb", bufs=2)
                    nc.gpsimd.partition_broadcast(rb, recip)
                    nc.vector.tensor_mul(o_n[0:64, qs], oacc0[0:64, :], rb[:, 0:512])
                    nc.vector.tensor_mul(
                        o_n[64:128, qs], oacc1[0:64, :], rb[:, 512:1024]
                    )

                def emit_tp(st):
                    ss = slice(st * 128, (st + 1) * 128)
                    tp = ps_misc.tile([128, 512], f32, tag="misc", name="tp")
                    nc.tensor.matmul(tp, o_n[:, ss], wo_sb, start=True, stop=True)
                    out_sb = work.tile([128, 512], f32, tag="outsb", bufs=4)
                    nc.vector.tensor_copy(out_sb, tp)
                    nc.sync.dma_start(out=out[ss, :], in_=out_sb)

                def emit_scores_exp(c, t, pt):
                    qs_c = slice(c * 512, (c + 1) * 512)
                    sc = ps_sc.tile([128, 1024], f32, tag="sc", name="sc")
                    nc.tensor.matmul(
                        sc[:, 0:512],
                        k_t[0:64, t * 128 : (t + 1) * 128],
                        q_t[0:64, qs_c],
                        start=True,
                        stop=True,
                    )
                    nc.tensor.matmul(
                        sc[:, 512:1024],
                        k_t[64:128, t * 128 : (t + 1) * 128],
                        q_t[64:128, qs_c],
                        start=True,
                        stop=True,
                    )
                    nc.scalar.activation(
                        out=pt[:, :, :],
                        in_=sc,
                        func=EXP,
                        scale=0.125,
                        bias=biasc_sb[:, t : t + 1],
                    )

                # ---- PE p-state warmup: dependency-free matmuls keep the
                # PE continuously busy from ~1.2us until the first projection
                # inputs land (~5.4us), so the projections and first scores
                # run at full clock instead of the 2x-slow ramp p-state ----
                warm_ps = ps_sc.tile([128, 1024], f32, tag="sc", name="warm")
                for _w in range(15):
                    nc.tensor.matmul(
                        warm_ps[0:64, 0:256],
                        ones_bf[:, 0:64],
                        ones_bf,
                        start=True,
                        stop=True,
                    )

                # ---- prologue: Q(0) + first K tile, queue the rest ----
                x1cur = x1c0 if _rep == 0 else x1_fetch(0)
                psq_box = [None]
                for kt in range(4):
                    emit_qproj_mm(0, x1cur, kt, psq_box, scalar_copy=True)
                psk0 = ps_misc.tile([128, 256], f32, tag="misc", name="psk0")
                for kt in range(4):
                    nc.tensor.matmul(
                        psk0,
                        wk_sb[:, kt, :],
                        x2all[:, kt, 0:256],
                        start=(kt == 0),
                        stop=(kt == 3),
                    )
                nc.vector.tensor_copy(k_t[:, 0:256], psk0)

                # deferred PE jobs: (min_slot, fn), drained in order once
                # t >= min_slot; <=2 per slot in chunk 0, <=1 afterwards
                miscq = []
                miscq.append((0, lambda: emit_kv_k(0, lo=256)))
                for kc in range(1, NKC):
                    bx = [None]
                    miscq.append((0, lambda kc=kc, bx=bx: emit_kv_k(kc, half=0, box=bx)))
                    miscq.append((0, lambda kc=kc, bx=bx: emit_kv_k(kc, half=1, box=bx)))
                for tt in range(0, 2):
                    miscq.append((0, lambda tt=tt: emit_kv_v(tt)))
                for tt in range(2, NT):
                    miscq.append((1, lambda tt=tt: emit_kv_v(tt)))

                prev_chunk = None  # (c, oacc0, oacc1) not yet normalized
                pending = []  # [(oacc0, oacc1, t, pt)] w/o AV emitted yet
                for c in range(NQC):
                    oacc0 = ps_oacc.tile([65, 512], f32, tag="oacc", name="oacc0")
                    oacc1 = ps_oacc.tile([65, 512], f32, tag="oacc", name="oacc1")
                    if c + 1 < NQC:
                        x1next = x1_fetch(c + 1)  # DMA out now, used at t>=11

                    for t in range(NT):
                        pt = pexp.tile([128, 2, 512], bf16, tag="pt", name="pt")
                        emit_scores_exp(c, t, pt)
                        pending.append((oacc0, oacc1, t, pt))
                        # finish the previous chunk's AVs in the first slots
                        dr = 0
                        while pending and pending[0][0] is not oacc0 and dr < 2:
                            emit_av(*pending.pop(0))
                            dr += 1
                        if t == 0 and prev_chunk is not None:
                            emit_norm(*prev_chunk)
                            nc_ = prev_chunk[0]
                            for st in range(4 * nc_, 4 * nc_ + 4):
                                miscq.append((7, lambda st=st: emit_tp(st)))
                            prev_chunk = None
                        if t == 2 and c + 1 < NQC:
                            pb = [None]
                            for kt in range(4):
                                miscq.append(
                                    (11, lambda kt=kt, c=c, x=x1next, pb=pb:
                                        emit_qproj_mm(c + 1, x, kt, pb))
                                )
                        # own-chunk AVs lag until the oacc WAR with the
                        # previous chunk's norm has cleared
                        if t >= 9:
                            lag = 1
                            dr = 0
                            while len(pending) > lag and dr < 2:
                                emit_av(*pending.pop(0))
                                dr += 1
                        # at most one deferred job per slot (two in chunk 0)
                        lim = 3 if c == 0 and t < 4 else (2 if c == 0 and t < 8 else 1)
                        if t == NT - 1:
                            lim = len(miscq)  # jobs must not cross the chunk
                        dj = 0
                        while miscq and miscq[0][0] <= t and dj < lim:
                            miscq.pop(0)[1]()
                            dj += 1
                    prev_chunk = (c, oacc0, oacc1)

                # ---- tail: drain, then final chunk norm + projection with a
                # PE broadcast (PE is idle here; skips the Pool launch+sems)
                while pending:
                    emit_av(*pending.pop(0))
                while miscq:
                    miscq.pop(0)[1]()
                # final-chunk norm: ScalarE is idle now -- oacc copies go on
                # the scalar engine in parallel with the DVE reciprocals, the
                # broadcast runs on the PE, and the four output-tile copies
                # alternate scalar/DVE.
                CPY = mybir.ActivationFunctionType.Copy
                cl, oacc0, oacc1 = prev_chunk
                qs = slice(cl * 512, (cl + 1) * 512)
                os0 = work.tile([64, 512], bf16, tag="osb", bufs=2)
                nc.scalar.activation(out=os0, in_=oacc0[0:64, :], func=CPY)
                os1 = work.tile([64, 512], bf16, tag="osb", bufs=2)
                nc.scalar.activation(out=os1, in_=oacc1[0:64, :], func=CPY)
                recipr = work.tile([1, 1024], f32r, tag="recipr")
                nc.vector.reciprocal(recipr[:, 0:512], oacc0[64:65, :])
                nc.vector.reciprocal(recipr[:, 512:1024], oacc1[64:65, :])
                rb0 = ps_misc.tile([128, 512], f32, tag="misc", name="rb0")
                nc.tensor.matmul(
                    rb0[0:64, :], ones64, recipr[:, 0:512], start=True, stop=True
                )
                rb1 = ps_misc.tile([128, 512], f32, tag="misc", name="rb1")
                nc.tensor.matmul(
                    rb1[0:64, :], ones64, recipr[:, 512:1024], start=True, stop=True
                )
                nc.vector.tensor_mul(o_n[0:64, qs], os0, rb0[0:64, :])
                nc.vector.tensor_mul(o_n[64:128, qs], os1, rb1[0:64, :])
                for j, st in enumerate(range(4 * cl, 4 * cl + 4)):
                    ss = slice(st * 128, (st + 1) * 128)
                    tp = ps_misc.tile([128, 512], f32, tag="misc", name="tp")
                    nc.tensor.matmul(tp, o_n[:, ss], wo_sb, start=True, stop=True)
                    # bf16 halves the final DMAs that gate the kernel's end;
                    # only these 512 rows take the extra rounding
                    ob = work.tile([128, 512], bf16, tag="outbf", bufs=4)
                    if j % 2 == 0:
                        nc.scalar.activation(out=ob, in_=tp, func=CPY)
                    else:
                        nc.vector.tensor_copy(ob, tp)
                    nc.sync.dma_start(
                        out=out_bf[j * 128 : (j + 1) * 128, :], in_=ob
                    )

    nc.compile()
    return nc


def _get_runtime(skc: int, reps: int = 1):
    key = (skc, reps)
    if key not in _RUNTIMES:
        _RUNTIMES[key] = _build_program(skc, reps)
    return _RUNTIMES[key]


def _numpy_reference(x1, x2, mask, Wq, bq, Wk, bk, Wv, bv, Wo, bo):
    q = (x1 @ Wq + bq).reshape(B, S, H, DH).transpose(0, 2, 1, 3)
    k = (x2 @ Wk + bk).reshape(B, S, H, DH).transpose(0, 2, 1, 3)
    v = (x2 @ Wv + bv).reshape(B, S, H, DH).transpose(0, 2, 1, 3)
    scores = np.einsum("bhqd,bhkd->bhqk", q, k) / np.sqrt(np.float32(DH))
    scores = scores + mask[:, None, None, :].astype(np.float32) * np.float32(-1e9)
    scores = scores - scores.max(axis=-1, keepdims=True)
    e = np.exp(scores)
    attn = e / e.sum(axis=-1, keepdims=True)
    o = np.einsum("bhqk,bhkd->bhqd", attn, v)
    o = o.transpose(0, 2, 1, 3).reshape(B, S, D)
    return (o @ Wo + bo).astype(np.float32)


def _make_in_maps(x1, x2, mask, Wq, Wk, Wv, Wo):
    bf = ml_dtypes.bfloat16
    keep = [np.nonzero(mask[b] == 0)[0] for b in range(B)]
    counts = [len(k) for k in keep]
    skc = ((max(counts) + 127) // 128) * 128
    nt = skc // 128
    in_maps = []
    for c in range(NCORES):
        b, hp = c // 4, c % 4
        x2c = np.zeros((skc, D), dtype=np.float32)
        x2c[: counts[b]] = x2[b][keep[b]]
        # exp bias column per key tile: -C_SHIFT valid keys, -30 padding
        bias = np.full((nt, 128), -30.0, dtype=np.float32)
        bias.reshape(-1)[: counts[b]] = -C_SHIFT
        cols = slice(hp * 128, (hp + 1) * 128)
        in_maps.append(
            {
                "x1t": np.ascontiguousarray(x1[b].T).astype(bf),
                "x2ct": np.ascontiguousarray(x2c.T).astype(bf),
                "biasc": np.ascontiguousarray(bias.T),
                "wqkv": np.ascontiguousarray(
                    np.concatenate([Wq[:, cols], Wk[:, cols], Wv[:, cols]], axis=1)
                ).astype(bf),
                "wo": np.ascontiguousarray(Wo[cols, :]).astype(bf),
            }
        )
    return skc, in_maps


def kernel(x1, x2, mask, Wq, bq, Wk, bk, Wv, bv, Wo, bo):
    from concourse.bass_utils import run_bass_kernel_spmd

    x1 = np.asarray(x1, dtype=np.float32)
    x2 = np.asarray(x2, dtype=np.float32)
    mask = np.asarray(mask)
    Wq = np.asarray(Wq, dtype=np.float32)
    Wk = np.asarray(Wk, dtype=np.float32)
    Wv = np.asarray(Wv, dtype=np.float32)
    Wo = np.asarray(Wo, dtype=np.float32)
    bq, bk, bv, bo = (np.asarray(b, dtype=np.float32) for b in (bq, bk, bv, bo))

    counts = [int((mask[b] == 0).sum()) for b in range(B)]
    if any(np.abs(b).max() > 0 for b in (bq, bk, bv) if b.size) or min(counts) == 0:
        return _numpy_reference(x1, x2, mask, Wq, bq, Wk, bk, Wv, bv, Wo, bo)

    skc, in_maps = _make_in_maps(x1, x2, mask, Wq, Wk, Wv, Wo)
    nc = _get_runtime(skc)

    res = run_bass_kernel_spmd(nc, in_maps, core_ids=list(range(NCORES)))
    full = np.empty((B, S, D), dtype=np.float32)
    for b in range(B):
        acc = res.results[4 * b]["out"].astype(np.float32, copy=True)
        accb = res.results[4 * b]["out_bf"].astype(np.float32)
        for hp in range(1, 4):
            acc += res.results[4 * b + hp]["out"]
            accb += res.results[4 * b + hp]["out_bf"].astype(np.float32)
        acc[S - 512 :] = accb
        full[b] = acc + bo
    return full
